# revision 7
# baseline (speedup 1.0000x reference)
"""Batch Child-Sum TreeLSTM cell on 8 Trainium2 NeuronCores.

Strategy (data-parallel over nodes; fp16 matmuls, PE-roofline ~191us/core):
  - Shard the N nodes (and their contiguous child segments) evenly across the
    8 cores; replicate the small weight matrices. Irregular sorted
    segment_ids are first regularized host-side by zero-padding every node to
    max_children slots (exact: padded slots contribute 0).
  - Host stages activations feature-major (features on SBUF partitions) and
    child-major (one contiguous slab per child slot), cast to fp16 (same PE
    rate as bf16, 10-bit mantissa - strictly more accurate for this O(5)
    data; fp8 DoubleRow was measured too coarse: ~4e-2 rel err vs the 2e-2
    gate, so fp16's 18 matmul-streams/node ~190us is the PE floor).
  - PSUM ping-pong: per 512-node sub, a 3-bank z-group [z_i|z_o|z_u] and a
    3-bank f-group [f0|f1|f2], each SINGLE-buffered but drained in opposite
    phases: ACT evacuates z(k) while the PE fills f(k), and f(k) while the
    PE fills z(k+1). ACT's drain (1.6/1.4us) is faster than each PE fill
    phase (1.9us), so nobody stalls - full-width FD=512 matmuls without the
    2x-PSUM-banks cost of double buffering (which doesn't fit: 24B/node of
    PSUM transit x 512 x 2 > 16KB).
  - ACT work drops ~192->~155us: one merged sigmoid for z_i|z_o (zero
    biases per the input spec; per-region-bias path kept), one for all 3
    f-slots, tanh(z_u); tanh(c) runs once per macro, SPLIT in half and
    injected into per-sub ACT idle slots so it never delays a PSUM drain.
  - DVE work drops ~181->~150us: the whole gate chain runs ONCE per 2048-
    node macro as wide fp16 2x-mode tensor ops (h_tilde child-sum, f*c,
    segment sums, c/h assembly) via strided 3D access patterns; per-op
    overhead (~100cyc) amortizes. The next macro's h_tilde is hoisted ahead
    of the current macro's chain in the in-order DVE queue so the PE never
    waits on it.
  - DMA: loads prefetch 2 macros deep (x/ch/cc bufs=3) to ride out the
    bursty 9MB-per-macro demand; outputs store fp16 on the near-idle gpsimd
    SWDGE queue; host upcasts. Measured engine busy/core: PE ~196.6us
    (bound, >4us of idle gaps total), ACT ~179us, DMA ~173us/queue, DVE
    ~147us; HW exec ~224-230us vs ~231-248us for the previous kernel.
"""

from contextlib import ExitStack

import numpy as np

import concourse.bass as bass
import concourse.bacc as bacc
import concourse.tile as tile
from concourse import mybir
from concourse.bass_utils import run_bass_kernel_spmd

F32 = mybir.dt.float32
FP16 = mybir.dt.float16

N_CORES = 8

# Tiling (in nodes). SUB: PSUM group width (3 regions x 512 f32 = 3 banks
# per group, two groups ping-ponged). MACRO: DMA / SBUF / DVE-chain
# granularity.
SUB = 512
MACRO = 2048


def _chunks(total, step):
    out = []
    off = 0
    while off < total:
        out.append((off, min(step, total - off)))
        off += step
    return out


def build_program(npc, in_dim, hid, cpn, zero_bias=True):
    """Bass program for one core's shard: npc nodes, npc*cpn edges."""
    assert in_dim == 256 and hid == 128
    assert npc % 512 == 0
    assert 1 <= cpn <= 5, "PSUM layout fits at most 5 child slots"

    nc = bacc.Bacc("TRN2", target_bir_lowering=False, debug=False)

    xT = nc.dram_tensor("xT", [hid, 2 * npc], FP16, kind="ExternalInput").ap()
    ch = nc.dram_tensor("ch", [hid, cpn * npc], FP16, kind="ExternalInput").ap()
    cc = nc.dram_tensor("cc", [hid, cpn * npc], FP16, kind="ExternalInput").ap()
    # all fp16 weights packed in one tensor: [wcx | wch | wfd | uf]
    WALL = 2 * 3 * hid + 3 * hid + 2 * hid + hid
    wall = nc.dram_tensor("wall", [hid, WALL], FP16, kind="ExternalInput").ap()
    bc3 = nc.dram_tensor("bc3", [hid, 3], F32, kind="ExternalInput").ap()
    bf1 = nc.dram_tensor("bf1", [hid, 1], F32, kind="ExternalInput").ap()

    cT = nc.dram_tensor("cT", [hid, npc], FP16, kind="ExternalOutput").ap()
    hT = nc.dram_tensor("hT", [hid, npc], FP16, kind="ExternalOutput").ap()

    xT3 = xT.rearrange("p (i n) -> p i n", i=2)
    ch3 = ch.rearrange("p (c n) -> p c n", c=cpn)
    cc3 = cc.rearrange("p (c n) -> p c n", c=cpn)

    ACTF = mybir.ActivationFunctionType

    with tile.TileContext(nc) as tc, ExitStack() as ctx:
        consts = ctx.enter_context(tc.tile_pool(name="consts", bufs=1))
        macro_pool = ctx.enter_context(tc.tile_pool(name="macro", bufs=2))
        tail_pool = ctx.enter_context(tc.tile_pool(name="tail", bufs=1))
        psum = ctx.enter_context(tc.tile_pool(name="psum", bufs=1, space="PSUM"))

        # ---- weights (resident, one DMA) ----
        wall_sb = consts.tile([128, WALL], FP16, tag="wall")
        nc.sync.dma_start(out=wall_sb, in_=wall)
        o = 0
        wcx3 = wall_sb[:, o : o + 2 * 3 * hid].rearrange("p (i m) -> p i m", i=2)
        o += 2 * 3 * hid
        wch_sb = wall_sb[:, o : o + 3 * hid]
        o += 3 * hid
        wfd3 = wall_sb[:, o : o + 2 * hid].rearrange("p (i m) -> p i m", i=2)
        o += 2 * hid
        uf_sb = wall_sb[:, o : o + hid]
        if not zero_bias:
            bc_sb = consts.tile([128, 3], F32, tag="bc3")
            nc.sync.dma_start(out=bc_sb, in_=bc3)
            bf_sb = consts.tile([128, 1], F32, tag="bf1")
            nc.sync.dma_start(out=bf_sb, in_=bf1)

        # ping-pong PSUM groups (single-buffered; the phase-offset schedule
        # keeps PE and ACT off each other's group). z_u gets its own
        # double-buffered bank so a lagging tanh(z_u) never blocks the PE's
        # next z-phase on the z-group WAR.
        zt = psum.tile([128, 2 * SUB], F32, tag="zt")
        ft = psum.tile([128, cpn * SUB], F32, tag="ft")

        # PE warmup: the HAM clock-gates an idle PE and opens to full rate
        # only after ~2.7us of sustained activity (measured: without this,
        # the first ~14 real matmuls ran at ~2x duration). Burn exactly that
        # window on dummy matmuls into the spare PSUM bank while the first
        # input DMAs land - no more, or the dummies delay real work.
        warm_sb = consts.tile([128, 512], FP16, tag="warm")
        nc.vector.memset(warm_sb, 0.0)
        warm_ps = psum.tile([128, 512], F32, tag="warmp")
        for _ in range(7):
            nc.tensor.matmul(
                warm_ps, lhsT=warm_sb[:, 0:128], rhs=warm_sb, start=True, stop=True
            )

        if npc > 2 * MACRO + 6144:
            # taper up (compute starts early, DMA builds lead); the last
            # macro runs its gate chain per-sub ("fine") so only ~4us of
            # epilogue trails the final matmul
            body = npc - 4608 - 1536
            macro_plan = [(0, 512), (512, 1024), (1536, 1536), (3072, 1536)]
            macro_plan += [(4608 + o, s) for o, s in _chunks(body, MACRO)]
            macro_plan.append((npc - 1536, 1536))
        else:
            macro_plan = _chunks(npc, MACRO)
        nm = len(macro_plan)

        def issue_dmas(m0, msz):
            x_t = macro_pool.tile([128, 2 * msz], FP16, tag="x", bufs=3)
            nc.sync.dma_start(
                out=x_t.rearrange("p (i n) -> p i n", i=2),
                in_=xT3[:, :, m0 : m0 + msz],
            )
            ch_t = macro_pool.tile([128, cpn * msz], FP16, tag="ch", bufs=3)
            nc.sync.dma_start(
                out=ch_t.rearrange("p (c n) -> p c n", c=cpn),
                in_=ch3[:, :, m0 : m0 + msz],
            )
            cc_t = macro_pool.tile([128, cpn * msz], FP16, tag="cc", bufs=3)
            nc.sync.dma_start(
                out=cc_t.rearrange("p (c n) -> p c n", c=cpn),
                in_=cc3[:, :, m0 : m0 + msz],
            )
            return x_t, ch_t, cc_t

        def issue_ht(tiles, msz):
            # h_tilde for a whole macro: wide DVE adds, issued a macro ahead
            # of use so the PE never waits behind the gate chain in the
            # in-order DVE queue
            cht3 = tiles[1].rearrange("p (c n) -> p c n", c=cpn)
            ht_t = macro_pool.tile([128, msz], FP16, tag="ht", bufs=3)
            nc.vector.tensor_add(ht_t, cht3[:, 0, :], cht3[:, 1, :])
            for ci in range(2, cpn):
                nc.vector.tensor_add(ht_t, ht_t, cht3[:, ci, :])
            return ht_t

        staged = {0: issue_dmas(*macro_plan[0])}
        if nm > 1:
            staged[1] = issue_dmas(*macro_plan[1])
        # macro 0 skips the DVE h_tilde: its z-phase accumulates the three
        # child slabs directly on the PE, so the ramp never waits on DVE
        hts = {0: None}
        prev = None  # (m0, msz, c_t, h_t, tc_t, sio2_t, nsub)

        for idx, (m0, msz) in enumerate(macro_plan):
            if idx + 2 < nm:
                staged[idx + 2] = issue_dmas(*macro_plan[idx + 2])
            if idx + 1 < nm:
                hts[idx + 1] = issue_ht(staged[idx + 1], macro_plan[idx + 1][1])
            fine = (idx == nm - 1) and (nm > 1)
            x_t, ch_t, cc_t = staged.pop(idx)
            ht_t = hts.pop(idx)
            xt3 = x_t.rearrange("p (i n) -> p i n", i=2)
            cht3 = ch_t.rearrange("p (c n) -> p c n", c=cpn)
            cct3 = cc_t.rearrange("p (c n) -> p c n", c=cpn)
            nsub = msz // SUB
            # per-sub sigmoid outputs, alive through next macro's injections
            sio2_t = macro_pool.tile([128, nsub * 2 * SUB], FP16, tag="sio2")
            fs_t = macro_pool.tile([128, nsub * cpn * SUB], FP16, tag="fs")
            tu_t = macro_pool.tile([128, msz], FP16, tag="tu")
            c_t = macro_pool.tile([128, msz], FP16, tag="c_out")
            h_t = macro_pool.tile([128, msz], FP16, tag="h_out")
            tc_t = macro_pool.tile([128, msz], FP16, tag="tanh_c")

            # previous macro's epilogue, injected into this macro's ACT/DVE
            # idle slots (tanh_c split in half so it never delays a drain)
            pending = []
            if prev is not None:
                pm0, pmsz, pc_t, ph_t, ptc_t, psio2, pnsub = prev
                half = (pmsz // 2 + SUB - 1) // SUB * SUB
                pending.append(
                    lambda: nc.scalar.activation(
                        ptc_t[:, :half], pc_t[:, :half], ACTF.Tanh
                    )
                )
                if half < pmsz:
                    pending.append(
                        lambda: nc.scalar.activation(
                            ptc_t[:, half:], pc_t[:, half:], ACTF.Tanh
                        )
                    )

                def _finish_prev():
                    pso3 = psio2.rearrange("p (k w) -> p k w", w=2 * SUB)[
                        :, :, SUB : 2 * SUB
                    ]
                    nc.vector.tensor_mul(
                        ph_t.rearrange("p (k n) -> p k n", n=SUB),
                        pso3,
                        ptc_t.rearrange("p (k n) -> p k n", n=SUB),
                    )
                    nc.gpsimd.dma_start(out=cT[:, pm0 : pm0 + pmsz], in_=pc_t)
                    nc.gpsimd.dma_start(out=hT[:, pm0 : pm0 + pmsz], in_=ph_t)

                pending.append(_finish_prev)

            if fine and nsub >= 2:
                # halve the last sub so the final serial epilogue is short
                sub_plan = [(i * SUB, SUB) for i in range(nsub - 1)]
                sub_plan += [
                    ((nsub - 1) * SUB, SUB // 2),
                    ((nsub - 1) * SUB + SUB // 2, SUB // 2),
                ]
            else:
                sub_plan = [(i * SUB, SUB) for i in range(nsub)]
            so_off = 0
            fs_off = 0
            for k, (s0, ssz) in enumerate(sub_plan):
                xs = xt3[:, :, s0 : s0 + ssz]
                htsl = None if ht_t is None else ht_t[:, s0 : s0 + ssz]
                zu_t = psum.tile([128, SUB], F32, tag="zu", bufs=2)
                zu = zu_t[:, 0:ssz]
                # z-phase: ALL SIX x-halves first (zi, zo, then zu), so the
                # sub never touches ch/ht until the x passes are done - the
                # ramp runs straight through the staggered DMA arrivals and
                # the h_tilde term lands last (never waiting on DVE)
                for j in range(2):
                    for i in range(2):
                        nc.tensor.matmul(
                            zt[:, j * SUB : j * SUB + ssz],
                            lhsT=wcx3[:, i, 128 * j : 128 * (j + 1)],
                            rhs=xs[:, i, :],
                            start=(i == 0),
                            stop=False,
                        )
                for i in range(2):
                    nc.tensor.matmul(
                        zu,
                        lhsT=wcx3[:, i, 256 : 256 + 128],
                        rhs=xs[:, i, :],
                        start=(i == 0),
                        stop=False,
                    )
                for j in range(2):
                    if ht_t is None:
                        for c in range(cpn):
                            nc.tensor.matmul(
                                zt[:, j * SUB : j * SUB + ssz],
                                lhsT=wch_sb[:, 128 * j : 128 * (j + 1)],
                                rhs=cht3[:, c, s0 : s0 + ssz],
                                start=False,
                                stop=(c == cpn - 1),
                            )
                    else:
                        nc.tensor.matmul(
                            zt[:, j * SUB : j * SUB + ssz],
                            lhsT=wch_sb[:, 128 * j : 128 * (j + 1)],
                            rhs=htsl,
                            start=False,
                            stop=True,
                        )
                sio2 = sio2_t[:, so_off : so_off + 2 * ssz]
                so_off += 2 * ssz
                zt3 = zt.rearrange("p (r n) -> p r n", r=2)[:, :, 0:ssz]
                sio23 = sio2.rearrange("p (r n) -> p r n", r=2)
                if zero_bias:
                    nc.scalar.activation(sio23, zt3, ACTF.Sigmoid)
                else:
                    nc.scalar.activation(
                        sio2[:, 0:ssz], zt[:, 0:ssz], ACTF.Sigmoid,
                        bias=bc_sb[:, 0:1],
                    )
                    nc.scalar.activation(
                        sio2[:, ssz : 2 * ssz], zt[:, SUB : SUB + ssz], ACTF.Sigmoid,
                        bias=bc_sb[:, 1:2],
                    )
                if ht_t is None:
                    for c in range(cpn):
                        nc.tensor.matmul(
                            zu, lhsT=wch_sb[:, 256 : 256 + 128],
                            rhs=cht3[:, c, s0 : s0 + ssz],
                            start=False, stop=(c == cpn - 1),
                        )
                else:
                    nc.tensor.matmul(
                        zu, lhsT=wch_sb[:, 256 : 256 + 128], rhs=htsl,
                        start=False, stop=True,
                    )
                nc.scalar.activation(
                    tu_t[:, s0 : s0 + ssz], zu, ACTF.Tanh,
                    **({} if zero_bias else {"bias": bc_sb[:, 2:3]}),
                )
                # f-phase: forget gates, child-major: f_c = U_f h_c + W_f x
                for c in range(cpn):
                    out = ft[:, c * SUB : c * SUB + ssz]
                    nc.tensor.matmul(
                        out,
                        lhsT=uf_sb,
                        rhs=cht3[:, c, s0 : s0 + ssz],
                        start=True,
                        stop=False,
                    )
                    for i in range(2):
                        nc.tensor.matmul(
                            out, lhsT=wfd3[:, i, :], rhs=xs[:, i, :],
                            start=False, stop=(i == 1),
                        )
                fs = fs_t[:, fs_off : fs_off + cpn * ssz]
                fs_off += cpn * ssz
                nc.scalar.activation(
                    fs.rearrange("p (r n) -> p r n", r=cpn),
                    ft.rearrange("p (r n) -> p r n", r=cpn)[:, :, 0:ssz],
                    ACTF.Sigmoid,
                    **({} if zero_bias else {"bias": bf_sb[:, 0:1]}),
                )
                # drip the previous macro's epilogue into the slack
                if pending and (k >= 1 or len(sub_plan) == 1):
                    pending.pop(0)()
                if fine:
                    # tail macro: finish nodes per-sub so almost nothing
                    # trails the last matmul; stores ride the fast HWDGE
                    # queue (no loads remain to head-of-line block)
                    fjc_s = tail_pool.tile([128, cpn * SUB], FP16, tag="fjcS")
                    fj3 = fjc_s[:, 0 : cpn * ssz].rearrange("p (c n) -> p c n", c=cpn)
                    for c in range(cpn):
                        nc.vector.tensor_mul(
                            fj3[:, c, :],
                            fs[:, c * ssz : (c + 1) * ssz],
                            cct3[:, c, s0 : s0 + ssz],
                        )
                    if cpn == 1:
                        fc_s = fjc_s[:, 0:ssz]
                    else:
                        fcs_t = tail_pool.tile([128, SUB], FP16, tag="fcS")
                        fc_s = fcs_t[:, 0:ssz]
                        nc.vector.tensor_add(fc_s, fj3[:, 0, :], fj3[:, 1, :])
                        for ci2 in range(2, cpn):
                            nc.vector.tensor_add(fc_s, fc_s, fj3[:, ci2, :])
                    csl = c_t[:, s0 : s0 + ssz]
                    nc.vector.tensor_mul(csl, sio2[:, 0:ssz], tu_t[:, s0 : s0 + ssz])
                    nc.vector.tensor_add(csl, csl, fc_s)
                    nc.sync.dma_start(out=cT[:, m0 + s0 : m0 + s0 + ssz], in_=csl)
                    tcsl = tc_t[:, s0 : s0 + ssz]
                    nc.scalar.activation(tcsl, csl, ACTF.Tanh)
                    hsl = h_t[:, s0 : s0 + ssz]
                    nc.vector.tensor_mul(hsl, sio2[:, ssz : 2 * ssz], tcsl)
                    nc.sync.dma_start(out=hT[:, m0 + s0 : m0 + s0 + ssz], in_=hsl)
            while pending:
                pending.pop(0)()
            if fine:
                prev = None
                continue

            # ---- whole-macro gate chain on DVE (wide 2x-mode fp16 ops) ----
            fsM = fs_t.rearrange("p (k w) -> p k w", w=cpn * SUB)
            ccM = cc_t.rearrange("p (c k n) -> p c k n", c=cpn, n=SUB)
            fjc_t = tail_pool.tile([128, cpn * msz], FP16, tag="fjc")
            fjc3 = fjc_t.rearrange("p (c n) -> p c n", c=cpn)
            for c in range(cpn):
                nc.vector.tensor_mul(
                    fjc3[:, c, :].rearrange("p (k n) -> p k n", n=SUB),
                    fsM[:, :, c * SUB : (c + 1) * SUB],
                    ccM[:, c],
                )
            if cpn == 1:
                fc_t = fjc_t
            else:
                fc_t = tail_pool.tile([128, msz], FP16, tag="fc")
                nc.vector.tensor_add(fc_t, fjc3[:, 0, :], fjc3[:, 1, :])
                for ci in range(2, cpn):
                    nc.vector.tensor_add(fc_t, fc_t, fjc3[:, ci, :])
            si3 = sio2_t.rearrange("p (k w) -> p k w", w=2 * SUB)[:, :, 0:SUB]
            c3 = c_t.rearrange("p (k n) -> p k n", n=SUB)
            nc.vector.tensor_mul(c3, si3, tu_t.rearrange("p (k n) -> p k n", n=SUB))
            nc.vector.tensor_add(c_t, c_t, fc_t)
            prev = (m0, msz, c_t, h_t, tc_t, sio2_t, nsub)

        # epilogue for the last macro (unless the fine tail already did it)
        if prev is not None:
            pm0, pmsz, pc_t, ph_t, ptc_t, psio2, pnsub = prev
            nc.scalar.activation(ptc_t, pc_t, ACTF.Tanh)
            pso3 = psio2.rearrange("p (k w) -> p k w", w=2 * SUB)[:, :, SUB : 2 * SUB]
            nc.vector.tensor_mul(
                ph_t.rearrange("p (k n) -> p k n", n=SUB),
                pso3,
                ptc_t.rearrange("p (k n) -> p k n", n=SUB),
            )
            nc.gpsimd.dma_start(out=cT[:, pm0 : pm0 + pmsz], in_=pc_t)
            nc.gpsimd.dma_start(out=hT[:, pm0 : pm0 + pmsz], in_=ph_t)

    nc.compile()
    return nc


TRACE = False  # set True (e.g. from test.py) to capture an NTFF profile
LAST_RESULTS = None  # BassKernelResults of the most recent kernel() call

_PROGRAM_CACHE = {}


def _get_program(npc, in_dim, hid, cpn, zero_bias):
    key = (npc, in_dim, hid, cpn, zero_bias, SUB, MACRO)
    if key not in _PROGRAM_CACHE:
        _PROGRAM_CACHE[key] = build_program(npc, in_dim, hid, cpn, zero_bias)
    return _PROGRAM_CACHE[key]


def _pad_children(child_c, child_h, segment_ids, n):
    """Regularize to exactly max_c children per node (zero padding is exact:
    padded slots contribute sigmoid(..)*0 to fc and 0 to the child sum)."""
    seg = np.asarray(segment_ids).astype(np.int64)
    e = seg.shape[0]
    counts = np.bincount(seg, minlength=n)
    max_c = int(counts.max()) if e else 1
    if e == n * max_c and np.all(counts == max_c):
        return child_c, child_h, max_c  # already regular (and sorted)
    hid = child_h.shape[1]
    slot = np.arange(e, dtype=np.int64) - np.repeat(
        np.concatenate([[0], np.cumsum(counts)[:-1]]), counts
    )
    cc = np.zeros((n * max_c, hid), np.float32)
    ch = np.zeros((n * max_c, hid), np.float32)
    idx = seg * max_c + slot
    cc[idx] = child_c
    ch[idx] = child_h
    return cc, ch, max_c


def _stage_weights(W_combined, W_f, U_f, b_combined, b_f, hid):
    Wc = np.asarray(W_combined, dtype=np.float32)
    wcx = Wc[: 2 * hid].reshape(2, hid, 3 * hid).transpose(1, 0, 2).reshape(
        hid, 2 * 3 * hid
    )
    wch = Wc[2 * hid :]
    Wf = np.asarray(W_f, dtype=np.float32)
    wfd = Wf.reshape(2, hid, hid).transpose(1, 0, 2).reshape(hid, 2 * hid)
    ufs = np.asarray(U_f, dtype=np.float32)
    wall = np.ascontiguousarray(
        np.concatenate([wcx, wch, wfd, ufs], axis=1).astype(np.float16)
    )
    bc3 = np.ascontiguousarray(
        np.asarray(b_combined, dtype=np.float32).reshape(3, hid).T
    )
    bf1 = np.ascontiguousarray(np.asarray(b_f, dtype=np.float32).reshape(hid, 1))
    return wall, bc3, bf1


def kernel(
    inputs,
    child_c,
    child_h,
    segment_ids,
    W_combined,
    b_combined,
    W_f,
    U_f,
    b_f,
):
    inputs = np.asarray(inputs, dtype=np.float32)
    child_c = np.asarray(child_c, dtype=np.float32)
    child_h = np.asarray(child_h, dtype=np.float32)
    n, in_dim = inputs.shape
    hid = U_f.shape[0]

    child_c, child_h, cpn = _pad_children(child_c, child_h, segment_ids, n)

    assert n % N_CORES == 0
    npc = n // N_CORES
    npp = ((npc + 511) // 512) * 512  # padded nodes per core

    zero_bias = not (np.any(b_combined) or np.any(b_f))
    nc = _get_program(npp, in_dim, hid, cpn, zero_bias)
    wall, bc3, bf1 = _stage_weights(W_combined, W_f, U_f, b_combined, b_f, hid)

    in_maps = []
    for c in range(N_CORES):
        n0, n1 = c * npc, (c + 1) * npc
        e0, e1 = n0 * cpn, n1 * cpn
        xpad = np.zeros((hid, 2, npp), np.float16)
        xpad[:, :, :npc] = inputs[n0:n1].reshape(npc, 2, hid).transpose(2, 1, 0)
        chpad = np.zeros((hid, cpn, npp), np.float16)
        chpad[:, :, :npc] = (
            child_h[e0:e1].reshape(npc, cpn, hid).transpose(2, 1, 0)
        )
        ccpad = np.zeros((hid, cpn, npp), np.float16)
        ccpad[:, :, :npc] = (
            child_c[e0:e1].reshape(npc, cpn, hid).transpose(2, 1, 0)
        )
        in_maps.append(
            {
                "xT": xpad.reshape(hid, 2 * npp),
                "ch": chpad.reshape(hid, cpn * npp),
                "cc": ccpad.reshape(hid, cpn * npp),
                "wall": wall,
                "bc3": bc3,
                "bf1": bf1,
            }
        )

    res = run_bass_kernel_spmd(
        nc, in_maps, core_ids=list(range(N_CORES)), trace=TRACE
    )
    global LAST_RESULTS
    LAST_RESULTS = res

    c_full = np.empty((n, hid), np.float32)
    h_full = np.empty((n, hid), np.float32)
    for c in range(N_CORES):
        n0, n1 = c * npc, (c + 1) * npc
        c_full[n0:n1] = res.results[c]["cT"][:, :npc].T.astype(np.float32)
        h_full[n0:n1] = res.results[c]["hT"][:, :npc].T.astype(np.float32)
    return (c_full, h_full)



# revision 10
# speedup vs baseline: 1.1309x; 1.1309x over previous
"""Batch Child-Sum TreeLSTM cell on 8 Trainium2 NeuronCores.

Strategy (data-parallel over nodes; fp16 matmuls, PE-roofline ~191us/core):
  - Shard the N nodes (and their contiguous child segments) evenly across the
    8 cores; replicate the small weight matrices. Irregular sorted
    segment_ids are first regularized host-side by zero-padding every node to
    max_children slots (exact: padded slots contribute 0).
  - Host stages activations feature-major (features on SBUF partitions) and
    child-major (one contiguous slab per child slot), cast to fp16 (same PE
    rate as bf16, 10-bit mantissa - strictly more accurate for this O(5)
    data; fp8 DoubleRow was measured too coarse: ~4e-2 rel err vs the 2e-2
    gate, so fp16's 18 matmul-streams/node ~190us is the PE floor).
  - PSUM ping-pong: per 512-node sub, a 3-bank z-group [z_i|z_o|z_u] and a
    3-bank f-group [f0|f1|f2], each SINGLE-buffered but drained in opposite
    phases: ACT evacuates z(k) while the PE fills f(k), and f(k) while the
    PE fills z(k+1). ACT's drain (1.6/1.4us) is faster than each PE fill
    phase (1.9us), so nobody stalls - full-width FD=512 matmuls without the
    2x-PSUM-banks cost of double buffering (which doesn't fit: 24B/node of
    PSUM transit x 512 x 2 > 16KB).
  - ACT work drops ~192->~155us: one merged sigmoid for z_i|z_o (zero
    biases per the input spec; per-region-bias path kept), one for all 3
    f-slots, tanh(z_u); tanh(c) runs once per macro, SPLIT in half and
    injected into per-sub ACT idle slots so it never delays a PSUM drain.
  - DVE work drops ~181->~150us: the whole gate chain runs ONCE per 2048-
    node macro as wide fp16 2x-mode tensor ops (h_tilde child-sum, f*c,
    segment sums, c/h assembly) via strided 3D access patterns; per-op
    overhead (~100cyc) amortizes. The next macro's h_tilde is hoisted ahead
    of the current macro's chain in the in-order DVE queue so the PE never
    waits on it.
  - DMA: loads prefetch 2 macros deep (x/ch/cc bufs=3) to ride out the
    bursty 9MB-per-macro demand; outputs store fp16 on the near-idle gpsimd
    SWDGE queue; host upcasts. Measured engine busy/core: PE ~196.6us
    (bound, >4us of idle gaps total), ACT ~179us, DMA ~173us/queue, DVE
    ~147us; HW exec ~224-230us vs ~231-248us for the previous kernel.
"""

from contextlib import ExitStack

import numpy as np

import concourse.bass as bass
import concourse.bacc as bacc
import concourse.tile as tile
from concourse import mybir
from concourse.bass_utils import run_bass_kernel_spmd

F32 = mybir.dt.float32
FP16 = mybir.dt.float16

N_CORES = 8

# Tiling (in nodes). SUB: PSUM group width (3 regions x 512 f32 = 3 banks
# per group, two groups ping-ponged). MACRO: DMA / SBUF / DVE-chain
# granularity.
SUB = 512
MACRO = 2048


def _chunks(total, step):
    out = []
    off = 0
    while off < total:
        out.append((off, min(step, total - off)))
        off += step
    return out


def build_program(npc, in_dim, hid, cpn, zero_bias=True):
    """Bass program for one core's shard: npc nodes, npc*cpn edges."""
    assert in_dim == 256 and hid == 128
    assert npc % 512 == 0
    assert 1 <= cpn <= 5, "PSUM layout fits at most 5 child slots"

    nc = bacc.Bacc("TRN2", target_bir_lowering=False, debug=False)

    xT = nc.dram_tensor("xT", [hid, 2 * npc], FP16, kind="ExternalInput").ap()
    ch = nc.dram_tensor("ch", [hid, cpn * npc], FP16, kind="ExternalInput").ap()
    cc = nc.dram_tensor("cc", [hid, cpn * npc], FP16, kind="ExternalInput").ap()
    # all fp16 weights packed in one tensor: [wcx | wch | wfd | uf]
    WALL = 2 * 3 * hid + 3 * hid + 2 * hid + hid
    wall = nc.dram_tensor("wall", [hid, WALL], FP16, kind="ExternalInput").ap()
    bc3 = nc.dram_tensor("bc3", [hid, 3], F32, kind="ExternalInput").ap()
    bf1 = nc.dram_tensor("bf1", [hid, 1], F32, kind="ExternalInput").ap()

    cT = nc.dram_tensor("cT", [hid, npc], FP16, kind="ExternalOutput").ap()
    hT = nc.dram_tensor("hT", [hid, npc], FP16, kind="ExternalOutput").ap()

    xT3 = xT.rearrange("p (i n) -> p i n", i=2)
    ch3 = ch.rearrange("p (c n) -> p c n", c=cpn)
    cc3 = cc.rearrange("p (c n) -> p c n", c=cpn)

    ACTF = mybir.ActivationFunctionType

    with tile.TileContext(nc) as tc, ExitStack() as ctx:
        consts = ctx.enter_context(tc.tile_pool(name="consts", bufs=1))
        macro_pool = ctx.enter_context(tc.tile_pool(name="macro", bufs=2))
        tail_pool = ctx.enter_context(tc.tile_pool(name="tail", bufs=1))
        psum = ctx.enter_context(tc.tile_pool(name="psum", bufs=1, space="PSUM"))

        # ---- weights (resident, one DMA) ----
        wall_sb = consts.tile([128, WALL], FP16, tag="wall")
        nc.sync.dma_start(out=wall_sb, in_=wall)
        o = 0
        wcx3 = wall_sb[:, o : o + 2 * 3 * hid].rearrange("p (i m) -> p i m", i=2)
        o += 2 * 3 * hid
        wch_sb = wall_sb[:, o : o + 3 * hid]
        o += 3 * hid
        wfd3 = wall_sb[:, o : o + 2 * hid].rearrange("p (i m) -> p i m", i=2)
        o += 2 * hid
        uf_sb = wall_sb[:, o : o + hid]
        if not zero_bias:
            bc_sb = consts.tile([128, 3], F32, tag="bc3")
            nc.sync.dma_start(out=bc_sb, in_=bc3)
            bf_sb = consts.tile([128, 1], F32, tag="bf1")
            nc.sync.dma_start(out=bf_sb, in_=bf1)

        # ping-pong PSUM groups (single-buffered; the phase-offset schedule
        # keeps PE and ACT off each other's group). z_u gets its own
        # double-buffered bank so a lagging tanh(z_u) never blocks the PE's
        # next z-phase on the z-group WAR.
        zt = psum.tile([128, 2 * SUB], F32, tag="zt")
        ft = psum.tile([128, cpn * SUB], F32, tag="ft")

        # PE warmup: the HAM clock-gates an idle PE and opens to full rate
        # only after ~2.7us of sustained activity (measured: without this,
        # the first ~14 real matmuls ran at ~2x duration). Burn exactly that
        # window on dummy matmuls into the spare PSUM bank while the first
        # input DMAs land - no more, or the dummies delay real work.
        warm_sb = consts.tile([128, 512], FP16, tag="warm")
        nc.vector.memset(warm_sb, 0.0)
        warm_ps = psum.tile([128, 512], F32, tag="warmp")
        for _ in range(7):
            nc.tensor.matmul(
                warm_ps, lhsT=warm_sb[:, 0:128], rhs=warm_sb, start=True, stop=True
            )

        if npc > 2 * MACRO + 6144:
            # taper up (compute starts early, DMA builds lead); the last
            # macro runs its gate chain per-sub ("fine") so only ~4us of
            # epilogue trails the final matmul
            body = npc - 4608 - 1024
            macro_plan = [(0, 512), (512, 1024), (1536, 1536), (3072, 1536)]
            macro_plan += [(4608 + o, s) for o, s in _chunks(body, MACRO)]
            macro_plan.append((npc - 1024, 1024))
        else:
            macro_plan = _chunks(npc, MACRO)
        nm = len(macro_plan)

        def issue_dmas(m0, msz):
            # ch first: its first consumer (the hoisted DVE h_tilde of the
            # NEXT macro) fires a full macro before x/cc are touched, and a
            # late ch blocks the in-order DVE queue behind it. cc last: only
            # the end-of-macro gate chain reads it.
            ch_t = macro_pool.tile([128, cpn * msz], FP16, tag="ch", bufs=4)
            nc.sync.dma_start(
                out=ch_t.rearrange("p (c n) -> p c n", c=cpn),
                in_=ch3[:, :, m0 : m0 + msz],
            )
            x_t = macro_pool.tile([128, 2 * msz], FP16, tag="x", bufs=3)
            nc.sync.dma_start(
                out=x_t.rearrange("p (i n) -> p i n", i=2),
                in_=xT3[:, :, m0 : m0 + msz],
            )
            cc_t = macro_pool.tile([128, cpn * msz], FP16, tag="cc", bufs=3)
            nc.sync.dma_start(
                out=cc_t.rearrange("p (c n) -> p c n", c=cpn),
                in_=cc3[:, :, m0 : m0 + msz],
            )
            return x_t, ch_t, cc_t

        def issue_ht(tiles, msz):
            # h_tilde for a whole macro: wide DVE adds, issued a macro ahead
            # of use so the PE never waits behind the gate chain in the
            # in-order DVE queue
            cht3 = tiles[1].rearrange("p (c n) -> p c n", c=cpn)
            ht_t = macro_pool.tile([128, msz], FP16, tag="ht", bufs=3)
            nc.vector.tensor_add(ht_t, cht3[:, 0, :], cht3[:, 1, :])
            for ci in range(2, cpn):
                nc.vector.tensor_add(ht_t, ht_t, cht3[:, ci, :])
            return ht_t

        staged = {0: issue_dmas(*macro_plan[0])}
        if nm > 1:
            staged[1] = issue_dmas(*macro_plan[1])
        # macro 0 skips the DVE h_tilde: its z-phase accumulates the three
        # child slabs directly on the PE, so the ramp never waits on DVE
        hts = {0: None}
        prev = None  # (m0, msz, c_t, h_t, tc_t, sio2_t, nsub)

        for idx, (m0, msz) in enumerate(macro_plan):
            if idx + 2 < nm:
                staged[idx + 2] = issue_dmas(*macro_plan[idx + 2])
            if idx + 1 < nm:
                hts[idx + 1] = issue_ht(staged[idx + 1], macro_plan[idx + 1][1])
            fine = (idx == nm - 1) and (nm > 1)
            x_t, ch_t, cc_t = staged.pop(idx)
            ht_t = hts.pop(idx)
            xt3 = x_t.rearrange("p (i n) -> p i n", i=2)
            cht3 = ch_t.rearrange("p (c n) -> p c n", c=cpn)
            cct3 = cc_t.rearrange("p (c n) -> p c n", c=cpn)
            nsub = msz // SUB
            # per-sub sigmoid outputs, alive through next macro's injections
            sio2_t = macro_pool.tile([128, nsub * 2 * SUB], FP16, tag="sio2")
            fs_t = macro_pool.tile([128, nsub * cpn * SUB], FP16, tag="fs")
            tu_t = macro_pool.tile([128, msz], FP16, tag="tu")
            c_t = macro_pool.tile([128, msz], FP16, tag="c_out")
            h_t = macro_pool.tile([128, msz], FP16, tag="h_out")
            tc_t = macro_pool.tile([128, msz], FP16, tag="tanh_c")

            # previous macro's epilogue, injected into this macro's ACT/DVE
            # idle slots (tanh_c split in half so it never delays a drain)
            pending = []
            if prev is not None:
                pm0, pmsz, pc_t, ph_t, ptc_t, psio2, pnsub = prev
                half = (pmsz // 2 + SUB - 1) // SUB * SUB
                pending.append(
                    lambda: nc.scalar.activation(
                        ptc_t[:, :half], pc_t[:, :half], ACTF.Tanh
                    )
                )
                if half < pmsz:
                    pending.append(
                        lambda: nc.scalar.activation(
                            ptc_t[:, half:], pc_t[:, half:], ACTF.Tanh
                        )
                    )

                def _finish_prev():
                    pso3 = psio2.rearrange("p (k w) -> p k w", w=2 * SUB)[
                        :, :, SUB : 2 * SUB
                    ]
                    nc.vector.tensor_mul(
                        ph_t.rearrange("p (k n) -> p k n", n=SUB),
                        pso3,
                        ptc_t.rearrange("p (k n) -> p k n", n=SUB),
                    )
                    nc.gpsimd.dma_start(out=cT[:, pm0 : pm0 + pmsz], in_=pc_t)
                    nc.gpsimd.dma_start(out=hT[:, pm0 : pm0 + pmsz], in_=ph_t)

                pending.append(_finish_prev)

            if fine and nsub >= 2:
                # halve the last sub so the final serial epilogue is short
                sub_plan = [(i * SUB, SUB) for i in range(nsub - 1)]
                sub_plan += [
                    ((nsub - 1) * SUB, SUB // 2),
                    ((nsub - 1) * SUB + SUB // 2, SUB // 2),
                ]
            else:
                sub_plan = [(i * SUB, SUB) for i in range(nsub)]
            so_off = 0
            fs_off = 0
            for k, (s0, ssz) in enumerate(sub_plan):
                xs = xt3[:, :, s0 : s0 + ssz]
                htsl = None if ht_t is None else ht_t[:, s0 : s0 + ssz]
                zu_t = psum.tile([128, SUB], F32, tag="zu", bufs=2)
                zu = zu_t[:, 0:ssz]
                # z-phase: ALL SIX x-halves first (zi, zo, then zu), so the
                # sub never touches ch/ht until the x passes are done - the
                # ramp runs straight through the staggered DMA arrivals and
                # the h_tilde term lands last (never waiting on DVE)
                for j in range(2):
                    for i in range(2):
                        nc.tensor.matmul(
                            zt[:, j * SUB : j * SUB + ssz],
                            lhsT=wcx3[:, i, 128 * j : 128 * (j + 1)],
                            rhs=xs[:, i, :],
                            start=(i == 0),
                            stop=False,
                        )
                for i in range(2):
                    nc.tensor.matmul(
                        zu,
                        lhsT=wcx3[:, i, 256 : 256 + 128],
                        rhs=xs[:, i, :],
                        start=(i == 0),
                        stop=False,
                    )
                for j in range(2):
                    if ht_t is None:
                        for c in range(cpn):
                            nc.tensor.matmul(
                                zt[:, j * SUB : j * SUB + ssz],
                                lhsT=wch_sb[:, 128 * j : 128 * (j + 1)],
                                rhs=cht3[:, c, s0 : s0 + ssz],
                                start=False,
                                stop=(c == cpn - 1),
                            )
                    else:
                        nc.tensor.matmul(
                            zt[:, j * SUB : j * SUB + ssz],
                            lhsT=wch_sb[:, 128 * j : 128 * (j + 1)],
                            rhs=htsl,
                            start=False,
                            stop=True,
                        )
                sio2 = sio2_t[:, so_off : so_off + 2 * ssz]
                so_off += 2 * ssz
                zt3 = zt.rearrange("p (r n) -> p r n", r=2)[:, :, 0:ssz]
                sio23 = sio2.rearrange("p (r n) -> p r n", r=2)
                if zero_bias:
                    nc.scalar.activation(sio23, zt3, ACTF.Sigmoid)
                else:
                    nc.scalar.activation(
                        sio2[:, 0:ssz], zt[:, 0:ssz], ACTF.Sigmoid,
                        bias=bc_sb[:, 0:1],
                    )
                    nc.scalar.activation(
                        sio2[:, ssz : 2 * ssz], zt[:, SUB : SUB + ssz], ACTF.Sigmoid,
                        bias=bc_sb[:, 1:2],
                    )
                if ht_t is None:
                    for c in range(cpn):
                        nc.tensor.matmul(
                            zu, lhsT=wch_sb[:, 256 : 256 + 128],
                            rhs=cht3[:, c, s0 : s0 + ssz],
                            start=False, stop=(c == cpn - 1),
                        )
                else:
                    nc.tensor.matmul(
                        zu, lhsT=wch_sb[:, 256 : 256 + 128], rhs=htsl,
                        start=False, stop=True,
                    )
                nc.scalar.activation(
                    tu_t[:, s0 : s0 + ssz], zu, ACTF.Tanh,
                    **({} if zero_bias else {"bias": bc_sb[:, 2:3]}),
                )
                # f-phase: forget gates, child-major: f_c = U_f h_c + W_f x
                for c in range(cpn):
                    out = ft[:, c * SUB : c * SUB + ssz]
                    nc.tensor.matmul(
                        out,
                        lhsT=uf_sb,
                        rhs=cht3[:, c, s0 : s0 + ssz],
                        start=True,
                        stop=False,
                    )
                    for i in range(2):
                        nc.tensor.matmul(
                            out, lhsT=wfd3[:, i, :], rhs=xs[:, i, :],
                            start=False, stop=(i == 1),
                        )
                fs = fs_t[:, fs_off : fs_off + cpn * ssz]
                fs_off += cpn * ssz
                nc.scalar.activation(
                    fs.rearrange("p (r n) -> p r n", r=cpn),
                    ft.rearrange("p (r n) -> p r n", r=cpn)[:, :, 0:ssz],
                    ACTF.Sigmoid,
                    **({} if zero_bias else {"bias": bf_sb[:, 0:1]}),
                )
                # drip the previous macro's epilogue into the slack
                if pending and (k >= 1 or len(sub_plan) == 1):
                    pending.pop(0)()
                if fine:
                    # tail macro: finish nodes per-sub so almost nothing
                    # trails the last matmul; stores ride the fast HWDGE
                    # queue (no loads remain to head-of-line block)
                    fjc_s = tail_pool.tile([128, cpn * SUB], FP16, tag="fjcS")
                    fj3 = fjc_s[:, 0 : cpn * ssz].rearrange("p (c n) -> p c n", c=cpn)
                    for c in range(cpn):
                        nc.vector.tensor_mul(
                            fj3[:, c, :],
                            fs[:, c * ssz : (c + 1) * ssz],
                            cct3[:, c, s0 : s0 + ssz],
                        )
                    if cpn == 1:
                        fc_s = fjc_s[:, 0:ssz]
                    else:
                        fcs_t = tail_pool.tile([128, SUB], FP16, tag="fcS")
                        fc_s = fcs_t[:, 0:ssz]
                        nc.vector.tensor_add(fc_s, fj3[:, 0, :], fj3[:, 1, :])
                        for ci2 in range(2, cpn):
                            nc.vector.tensor_add(fc_s, fc_s, fj3[:, ci2, :])
                    csl = c_t[:, s0 : s0 + ssz]
                    nc.vector.tensor_mul(csl, sio2[:, 0:ssz], tu_t[:, s0 : s0 + ssz])
                    nc.vector.tensor_add(csl, csl, fc_s)
                    nc.sync.dma_start(out=cT[:, m0 + s0 : m0 + s0 + ssz], in_=csl)
                    tcsl = tc_t[:, s0 : s0 + ssz]
                    nc.scalar.activation(tcsl, csl, ACTF.Tanh)
                    hsl = h_t[:, s0 : s0 + ssz]
                    nc.vector.tensor_mul(hsl, sio2[:, ssz : 2 * ssz], tcsl)
                    nc.sync.dma_start(out=hT[:, m0 + s0 : m0 + s0 + ssz], in_=hsl)
            while pending:
                pending.pop(0)()
            if fine:
                prev = None
                continue

            # ---- whole-macro gate chain on DVE (wide 2x-mode fp16 ops) ----
            # f*c overwrites the cc tile in place (cc's last use), and the
            # child-sum accumulates into slab 0 - no separate fjc/fc tiles,
            # freeing SBUF for the deeper ch prefetch.
            fsM = fs_t.rearrange("p (k w) -> p k w", w=cpn * SUB)
            ccM = cc_t.rearrange("p (c k n) -> p c k n", c=cpn, n=SUB)
            fjc3 = cc_t.rearrange("p (c n) -> p c n", c=cpn)
            for c in range(cpn):
                nc.vector.tensor_mul(
                    ccM[:, c],
                    fsM[:, :, c * SUB : (c + 1) * SUB],
                    ccM[:, c],
                )
            fc_t = fjc3[:, 0, :]
            for ci in range(1, cpn):
                nc.vector.tensor_add(fc_t, fc_t, fjc3[:, ci, :])
            si3 = sio2_t.rearrange("p (k w) -> p k w", w=2 * SUB)[:, :, 0:SUB]
            c3 = c_t.rearrange("p (k n) -> p k n", n=SUB)
            nc.vector.tensor_mul(c3, si3, tu_t.rearrange("p (k n) -> p k n", n=SUB))
            nc.vector.tensor_add(c_t, c_t, fc_t)
            prev = (m0, msz, c_t, h_t, tc_t, sio2_t, nsub)

        # epilogue for the last macro (unless the fine tail already did it)
        if prev is not None:
            pm0, pmsz, pc_t, ph_t, ptc_t, psio2, pnsub = prev
            nc.scalar.activation(ptc_t, pc_t, ACTF.Tanh)
            pso3 = psio2.rearrange("p (k w) -> p k w", w=2 * SUB)[:, :, SUB : 2 * SUB]
            nc.vector.tensor_mul(
                ph_t.rearrange("p (k n) -> p k n", n=SUB),
                pso3,
                ptc_t.rearrange("p (k n) -> p k n", n=SUB),
            )
            nc.gpsimd.dma_start(out=cT[:, pm0 : pm0 + pmsz], in_=pc_t)
            nc.gpsimd.dma_start(out=hT[:, pm0 : pm0 + pmsz], in_=ph_t)

    nc.compile()
    return nc


TRACE = False  # set True (e.g. from test.py) to capture an NTFF profile
LAST_RESULTS = None  # BassKernelResults of the most recent kernel() call

_PROGRAM_CACHE = {}


def _get_program(npc, in_dim, hid, cpn, zero_bias):
    key = (npc, in_dim, hid, cpn, zero_bias, SUB, MACRO)
    if key not in _PROGRAM_CACHE:
        _PROGRAM_CACHE[key] = build_program(npc, in_dim, hid, cpn, zero_bias)
    return _PROGRAM_CACHE[key]


def _pad_children(child_c, child_h, segment_ids, n):
    """Regularize to exactly max_c children per node (zero padding is exact:
    padded slots contribute sigmoid(..)*0 to fc and 0 to the child sum)."""
    seg = np.asarray(segment_ids).astype(np.int64)
    e = seg.shape[0]
    counts = np.bincount(seg, minlength=n)
    max_c = int(counts.max()) if e else 1
    if e == n * max_c and np.all(counts == max_c):
        return child_c, child_h, max_c  # already regular (and sorted)
    hid = child_h.shape[1]
    slot = np.arange(e, dtype=np.int64) - np.repeat(
        np.concatenate([[0], np.cumsum(counts)[:-1]]), counts
    )
    cc = np.zeros((n * max_c, hid), np.float32)
    ch = np.zeros((n * max_c, hid), np.float32)
    idx = seg * max_c + slot
    cc[idx] = child_c
    ch[idx] = child_h
    return cc, ch, max_c


def _stage_weights(W_combined, W_f, U_f, b_combined, b_f, hid):
    Wc = np.asarray(W_combined, dtype=np.float32)
    wcx = Wc[: 2 * hid].reshape(2, hid, 3 * hid).transpose(1, 0, 2).reshape(
        hid, 2 * 3 * hid
    )
    wch = Wc[2 * hid :]
    Wf = np.asarray(W_f, dtype=np.float32)
    wfd = Wf.reshape(2, hid, hid).transpose(1, 0, 2).reshape(hid, 2 * hid)
    ufs = np.asarray(U_f, dtype=np.float32)
    wall = np.ascontiguousarray(
        np.concatenate([wcx, wch, wfd, ufs], axis=1).astype(np.float16)
    )
    bc3 = np.ascontiguousarray(
        np.asarray(b_combined, dtype=np.float32).reshape(3, hid).T
    )
    bf1 = np.ascontiguousarray(np.asarray(b_f, dtype=np.float32).reshape(hid, 1))
    return wall, bc3, bf1


def kernel(
    inputs,
    child_c,
    child_h,
    segment_ids,
    W_combined,
    b_combined,
    W_f,
    U_f,
    b_f,
):
    inputs = np.asarray(inputs, dtype=np.float32)
    child_c = np.asarray(child_c, dtype=np.float32)
    child_h = np.asarray(child_h, dtype=np.float32)
    n, in_dim = inputs.shape
    hid = U_f.shape[0]

    child_c, child_h, cpn = _pad_children(child_c, child_h, segment_ids, n)

    assert n % N_CORES == 0
    npc = n // N_CORES
    npp = ((npc + 511) // 512) * 512  # padded nodes per core

    zero_bias = not (np.any(b_combined) or np.any(b_f))
    nc = _get_program(npp, in_dim, hid, cpn, zero_bias)
    wall, bc3, bf1 = _stage_weights(W_combined, W_f, U_f, b_combined, b_f, hid)

    in_maps = []
    for c in range(N_CORES):
        n0, n1 = c * npc, (c + 1) * npc
        e0, e1 = n0 * cpn, n1 * cpn
        xpad = np.zeros((hid, 2, npp), np.float16)
        xpad[:, :, :npc] = inputs[n0:n1].reshape(npc, 2, hid).transpose(2, 1, 0)
        chpad = np.zeros((hid, cpn, npp), np.float16)
        chpad[:, :, :npc] = (
            child_h[e0:e1].reshape(npc, cpn, hid).transpose(2, 1, 0)
        )
        ccpad = np.zeros((hid, cpn, npp), np.float16)
        ccpad[:, :, :npc] = (
            child_c[e0:e1].reshape(npc, cpn, hid).transpose(2, 1, 0)
        )
        in_maps.append(
            {
                "xT": xpad.reshape(hid, 2 * npp),
                "ch": chpad.reshape(hid, cpn * npp),
                "cc": ccpad.reshape(hid, cpn * npp),
                "wall": wall,
                "bc3": bc3,
                "bf1": bf1,
            }
        )

    res = run_bass_kernel_spmd(
        nc, in_maps, core_ids=list(range(N_CORES)), trace=TRACE
    )
    global LAST_RESULTS
    LAST_RESULTS = res

    c_full = np.empty((n, hid), np.float32)
    h_full = np.empty((n, hid), np.float32)
    for c in range(N_CORES):
        n0, n1 = c * npc, (c + 1) * npc
        c_full[n0:n1] = res.results[c]["cT"][:, :npc].T.astype(np.float32)
        h_full[n0:n1] = res.results[c]["hT"][:, :npc].T.astype(np.float32)
    return (c_full, h_full)

